# revision 16
# baseline (speedup 1.0000x reference)
"""Bass/Tile TRN2 kernel for MultiHeadSelfAttention with 2D rope.

Strategy: data-parallel over batch (32 batches -> 4 per core x 8 cores).
All matmuls in fp16 on the PE (1 cycle/row, fp32 PSUM accumulate).
Layouts: activations kept transposed ([feature, token]) through qkv/attention
so every matmul contraction lands on the partition dim with zero on-chip
transposes except the initial x -> x^T.

Pipeline per core (tokens T = 4*577 = 2308):
  1. x [T,768] f32 -> cast fp16 -> PE-transpose -> xT [768,T]
  2. qkT = (W_qkv[:, :1536])^T-ish: out [1536 feat, T] via lhsT=W chunk,
     rhs=xT chunk; bias added on eviction (per-partition scalar)
  3. 2D rope applied in-place on qkT: rot = Pmat @ qkT (PE pair-swap),
     q' = q*cosT + rot*sinT (DVE/GPSIMD elementwise, precomputed tables)
  4. V in natural layout [T, 12*65] (64 cols/head + ones column for the
     softmax denominator), bias via K=1 ones-row matmul
  5. per (batch, head): scoresT [S_k, S_q] = k_sliceT.T @ q_sliceT;
     exp via ACT (scale=1/8 fused, no max-subtraction needed: |logits|<~4);
     ctxT [65, S_q] = V_aug.T @ expT accumulated over S_k chunks
     (row 64 = softmax denominator); normalize: reciprocal + DMA
     partition-broadcast + multiply into ctx_catT [768, T]
  6. proj: out [T, 768] = ctx_catT chunks (stationary) x W_proj (moving),
     bias via ones-row matmul, f32 out.
"""

import os
import sys

for _p in ("/opt/trn_rl_repo",):
    if _p not in sys.path and os.path.isdir(_p):
        sys.path.insert(0, _p)

import numpy as np

import concourse.bass as bass
import concourse.tile as tile
from concourse import bacc, mybir
from concourse.bass_utils import run_bass_kernel_spmd
from concourse.masks import make_identity

F32 = mybir.dt.float32
F16 = mybir.dt.float16

B, S, E = 32, 577, 768
H, DH = 12, 64
E3 = 3 * E  # 2304
N_CORES = 8
BPC = B // N_CORES  # 4 batches per core
T = BPC * S  # 2308 tokens per core
SCALE = DH ** -0.5
PT = 24
NO_ROPE = 1
THETA = 10000.0

# token tiles (global, for x / out / proj): 18*128 + 4
S_TILES = [(i * 128, min(128, T - i * 128)) for i in range((T + 127) // 128)]
# per-batch S_k tiles: 4*128 + 65
KT_TILES = [(i * 128, min(128, S - i * 128)) for i in range((S + 127) // 128)]
# S_q column chunks within a batch (PSUM bank-aligned)
QC_CHUNKS = [(0, 512), (512, 65)]
# 512-wide token chunks (1 PSUM bank each) for qk projection + rope
QK_CHUNKS = [(i * 512, min(512, T - i * 512)) for i in range((T + 511) // 512)]


def _host_tables():
    """cos/sin rope tables in transposed, head-paired, batch-tiled layout
    [128, T], plus the pair-swap (rotate-half) matrix [128, 128]."""
    dim = DH // 2  # 32
    inv_freq = 1.0 / (THETA ** (np.arange(0, dim, 2, dtype=np.float64) / dim))
    t = np.arange(PT, dtype=np.float64)
    f = np.einsum("s,f->sf", t, inv_freq)
    f = np.repeat(f, 2, axis=-1)  # [24, 32]
    f2 = np.concatenate(
        [
            np.broadcast_to(f[:, None, :], (PT, PT, dim)),
            np.broadcast_to(f[None, :, :], (PT, PT, dim)),
        ],
        axis=-1,
    ).reshape(PT * PT, DH)  # [576, 64]
    cos_nat = np.ones((S, DH), np.float64)
    sin_nat = np.zeros((S, DH), np.float64)
    cos_nat[NO_ROPE:] = np.cos(f2)
    sin_nat[NO_ROPE:] = np.sin(f2)
    cosT = cos_nat.T  # [64, 577]
    sinT = sin_nat.T
    cos128 = np.concatenate([cosT, cosT], axis=0)  # [128, 577] (head pair)
    sin128 = np.concatenate([sinT, sinT], axis=0)
    cos_full = np.tile(cos128, (1, BPC)).astype(np.float16)  # [128, 2308]
    sin_full = np.tile(sin128, (1, BPC)).astype(np.float16)

    p64 = np.zeros((DH, DH), np.float16)
    for m in range(0, DH, 2):
        p64[m + 1, m] = -1.0  # rot[even m] = -in[m+1]
        p64[m, m + 1] = 1.0  # rot[odd m]  = +in[m-1]
    pmat = np.zeros((128, 128), np.float16)
    pmat[:64, :64] = p64
    pmat[64:, 64:] = p64
    return cos_full, sin_full, pmat


def build_bass(n_iters: int = 1, phases: str = "wxvqap"):
    """Build the per-core bass program. n_iters>1 wraps the body in a
    hardware loop (identical recompute) for wall-clock timing.
    phases: subset of w(eights) x(transpose) v q(k+rope) a(ttn) p(roj),
    for timing knockouts only (outputs are garbage unless all enabled)."""
    nc = bacc.Bacc("TRN2", target_bir_lowering=False, debug=False)

    # x / W_qkv / W_proj are host-cast to fp16 (they are consumed in fp16
    # on-chip anyway) -- halves the DMA bytes and kills the cast stages.
    x_d = nc.dram_tensor("x", [T, E], F16, kind="ExternalInput").ap()
    wqkv_d = nc.dram_tensor("wqkv", [E, E3], F16, kind="ExternalInput").ap()
    bqkv_d = nc.dram_tensor("bqkv", [E3], F32, kind="ExternalInput").ap()
    bv16_d = nc.dram_tensor("bv16", [1, E], F16, kind="ExternalInput").ap()
    wproj_d = nc.dram_tensor("wproj", [E, E], F16, kind="ExternalInput").ap()
    bp16_d = nc.dram_tensor("bp16", [1, E], F16, kind="ExternalInput").ap()
    cos_d = nc.dram_tensor("cosT", [128, T], F16, kind="ExternalInput").ap()
    sin_d = nc.dram_tensor("sinT", [128, T], F16, kind="ExternalInput").ap()
    pmat_d = nc.dram_tensor("pmat", [128, 128], F16, kind="ExternalInput").ap()
    out_d = nc.dram_tensor("out", [T, E], F32, kind="ExternalOutput").ap()

    with tile.TileContext(nc) as tc:
        from contextlib import ExitStack

        with ExitStack() as top:
            if n_iters > 1:
                top.enter_context(tc.For_i(0, n_iters, 1))
            _emit_body(
                tc, top, x_d, wqkv_d, bqkv_d, bv16_d, wproj_d, bp16_d,
                cos_d, sin_d, pmat_d, out_d, phases,
            )
    nc.compile()
    return nc


def _emit_body(tc, ctx, x_d, wqkv_d, bqkv_d, bv16_d, wproj_d, bp16_d,
               cos_d, sin_d, pmat_d, out_d, phases="wxvqap"):
    nc = tc.nc
    AF = mybir.ActivationFunctionType
    OP = mybir.AluOpType

    # ---------------- long-lived pools ----------------
    consts = ctx.enter_context(tc.tile_pool(name="consts", bufs=1))
    qk16p = ctx.enter_context(tc.tile_pool(name="qk16", bufs=1))
    v16p = ctx.enter_context(tc.tile_pool(name="v16", bufs=1))
    ctx16p = ctx.enter_context(tc.tile_pool(name="ctx16", bufs=1))

    ident = consts.tile([128, 128], F16, tag="ident")
    make_identity(nc, ident)
    ones16 = consts.tile([1, 128], F16, tag="ones16")
    nc.vector.memset(ones16, 1.0)
    cosT = consts.tile([128, T], F16, tag="cosT")
    nc.sync.dma_start(cosT, cos_d)
    sinT = consts.tile([128, T], F16, tag="sinT")
    nc.sync.dma_start(sinT, sin_d)
    pmat = consts.tile([128, 128], F16, tag="pmat")
    nc.sync.dma_start(pmat, pmat_d)
    # b_qkv first 1536 features laid out [128, 12] (partition = feat % 128)
    bqk = consts.tile([128, 12], F32, tag="bqk")
    nc.sync.dma_start(
        bqk, bqkv_d[: 12 * 128].rearrange("(t p) -> p t", p=128)
    )
    bv16 = consts.tile([1, E], F16, tag="bv16")
    nc.sync.dma_start(bv16, bv16_d)
    bp16 = consts.tile([1, E], F16, tag="bp16")
    nc.sync.dma_start(bp16, bp16_d)

    qk16 = [qk16p.tile([128, T], F16, tag=f"qk{i}", name=f"qk{i}") for i in range(12)]
    v16 = [
        v16p.tile([128, H, DH + 1], F16, tag=f"v{b}_{kt}", name=f"v{b}_{kt}")
        for b in range(BPC)
        for kt in range(len(KT_TILES))
    ]
    ctx16 = [ctx16p.tile([128, T], F16, tag=f"ctx{i}", name=f"ctx{i}") for i in range(6)]

    with ExitStackCompat() as mid:
        wq16p = mid.enter_context(tc.tile_pool(name="wq16", bufs=1))
        xT16p = mid.enter_context(tc.tile_pool(name="xT16", bufs=1))
        wq16 = [wq16p.tile([128, E3], F16, tag=f"wq{i}", name=f"wq{i}") for i in range(6)]
        xT16 = [xT16p.tile([128, T], F16, tag=f"xT{i}", name=f"xT{i}") for i in range(6)]

        # ---------------- phase X: xT via DMA xbar transpose ----------------
        # x is f16 in DRAM; the xbar transpose engine writes xT directly,
        # freeing the PE (no 128x128 transposes) and the DVE (no copies)
        if "x" in phases:
            for ec in range(6):
                nc.sync.dma_start_transpose(
                    xT16[ec], x_d[:, ec * 128:(ec + 1) * 128]
                )

        # ---------------- phase W: load W_qkv (already f16) ----------------
        for ec in range(6 if "w" in phases else 0):
            nc.sync.dma_start(wq16[ec], wqkv_d[ec * 128:(ec + 1) * 128, :])

        # ---------------- phase V: V natural [T, 12*(64+1)] ----------------
        with tc.tile_pool(name="vpsum", bufs=4, space="PSUM") as vpsum:
            for b in range(BPC if "v" in phases else 0):
                for ikt, (koff, ksz) in enumerate(KT_TILES):
                    rows = b * S + koff
                    vt = v16[b * len(KT_TILES) + ikt]
                    pps = []
                    for half in range(2):
                        pp = vpsum.tile([128, 384], F32, tag="vp")
                        nc.tensor.matmul(
                            pp[:ksz],
                            ones16[:1, :ksz],
                            bv16[:1, half * 384:(half + 1) * 384],
                            start=True, stop=False,
                        )
                        pps.append(pp)
                    for ec in range(6):
                        for half in range(2):
                            nc.tensor.matmul(
                                pps[half][:ksz],
                                xT16[ec][:, rows:rows + ksz],
                                wq16[ec][:, 2 * E + half * 384:
                                         2 * E + (half + 1) * 384],
                                start=False, stop=(ec == 5),
                            )
                    for half in range(2):
                        nc.scalar.copy(
                            vt[:ksz, half * 6:(half + 1) * 6, :DH],
                            pps[half][:ksz].rearrange(
                                "p (h d) -> p h d", d=DH
                            ),
                        )
                    nc.vector.memset(vt[:ksz, :, DH:DH + 1], 1.0)

        # ---------------- phase QK: q/k transposed proj + rope ----------------
        with tc.tile_pool(name="qkpsum", bufs=6, space="PSUM") as qkpsum, \
             tc.tile_pool(name="rotpsum", bufs=2, space="PSUM") as rotpsum, \
             tc.tile_pool(name="ropetmp", bufs=4) as ropetmp:
            # interleave q and k chunks so attention can start early
            fc_order = [c for pair in zip(range(6), range(6, 12)) for c in pair]

            def emit_qk_mms(fc):
                # projection into PSUM (5x512-wide chunks), evict via ACT
                # (copy + per-partition bias), leaving DVE free for rope
                feat = fc * 128
                pps = []
                for (soff, slen) in QK_CHUNKS:
                    pp = qkpsum.tile([128, 512], F32, tag="qkp")
                    pps.append((pp, soff, slen))
                for ec in range(6):
                    for (pp, soff, slen) in pps:
                        nc.tensor.matmul(
                            pp[:, :slen],
                            wq16[ec][:, feat:feat + 128],
                            xT16[ec][:, soff:soff + slen],
                            start=(ec == 0), stop=(ec == 5),
                        )
                for (pp, soff, slen) in pps:
                    nc.scalar.activation(
                        qk16[fc][:, soff:soff + slen], pp[:, :slen],
                        AF.Identity, bias=bqk[:, fc:fc + 1],
                    )

            def emit_rope(fc):
                # rope, in place on qk16[fc]; elementwise all on DVE
                for (roff, rlen) in QK_CHUNKS:
                    rp = rotpsum.tile([128, 512], F32, tag="rotp")
                    nc.tensor.matmul(
                        rp[:, :rlen], pmat, qk16[fc][:, roff:roff + rlen],
                        start=True, stop=True,
                    )
                    rsin = ropetmp.tile([128, 512], F16, tag="rsin")
                    nc.vector.tensor_tensor(
                        rsin[:, :rlen], rp[:, :rlen],
                        sinT[:, roff:roff + rlen], op=OP.mult,
                    )
                    qcos = ropetmp.tile([128, 512], F16, tag="qcos")
                    nc.gpsimd.tensor_tensor(
                        qcos[:, :rlen], qk16[fc][:, roff:roff + rlen],
                        cosT[:, roff:roff + rlen], op=OP.mult,
                    )
                    nc.vector.tensor_tensor(
                        qk16[fc][:, roff:roff + rlen], qcos[:, :rlen],
                        rsin[:, :rlen], op=OP.add,
                    )

            if "q" in phases:
                prev_fc = None
                for fc in fc_order:
                    emit_qk_mms(fc)
                    if prev_fc is not None:
                        emit_rope(prev_fc)
                    prev_fc = fc
                emit_rope(prev_fc)

        # wq16 / xT16 freed here

    # ---------------- phase ATTN (software-pipelined emission) ----------------
    # Per (batch, head) pair: scores -> exp -> ctx -> normalize.  Emission is
    # pipelined one pair deep: ctx MMs for pair i-1 are issued AFTER the
    # scores MMs + exp ACTs of pair i, so the in-order PE queue always has
    # independent score work while ACT computes exp for the previous pair.
    with tc.tile_pool(name="spsum", bufs=2, space="PSUM") as spsum, \
         tc.tile_pool(name="cpsum", bufs=2, space="PSUM") as cpsum, \
         tc.tile_pool(name="e16p", bufs=12) as e16p, \
         tc.tile_pool(name="nrm", bufs=6) as nrm:
        pairs = [(b, 2 * j + o)
                 for j in range(6) for b in range(BPC) for o in range(2)]

        def emit_scores_exp(i):
            b, h = pairs[i]
            qc, kc, pr, bc0 = h // 2, 6 + h // 2, (h % 2) * 64, b * S
            etiles = []
            for (koff, ksz) in KT_TILES:
                sp = spsum.tile([128, S], F32, tag="sp")
                for (qoff, qlen) in QC_CHUNKS:
                    nc.tensor.matmul(
                        sp[:ksz, qoff:qoff + qlen],
                        qk16[kc][pr:pr + 64, bc0 + koff:bc0 + koff + ksz],
                        qk16[qc][pr:pr + 64, bc0 + qoff:bc0 + qoff + qlen],
                        start=True, stop=True,
                    )
                e = e16p.tile([128, S], F16, tag="e16")
                nc.scalar.activation(e[:ksz], sp[:ksz], AF.Exp, scale=SCALE)
                etiles.append(e)
            return etiles

        def emit_ctx(i, etiles):
            b, h = pairs[i]
            qc, pr, bc0 = h // 2, (h % 2) * 64, b * S
            cp = cpsum.tile([128, S], F32, tag="cp")
            for ikt, (koff, ksz) in enumerate(KT_TILES):
                vt = v16[b * len(KT_TILES) + ikt]
                for (qoff, qlen) in QC_CHUNKS:
                    nc.tensor.matmul(
                        cp[:DH + 1, qoff:qoff + qlen],
                        vt[:ksz, h, :],
                        etiles[ikt][:ksz, qoff:qoff + qlen],
                        start=(ikt == 0), stop=(ikt == len(KT_TILES) - 1),
                    )
            # normalize: ctx[:64] * (1 / denom row), partition-broadcast
            rec = nrm.tile([1, S], F16, tag="rec")
            with nc.allow_low_precision(reason="softmax denom recip, fp16 ok"):
                nc.vector.reciprocal(rec, cp[DH:DH + 1, :])
            bc = nrm.tile([64, S], F16, tag="bc")
            nc.gpsimd.partition_broadcast(bc, rec)
            nc.vector.tensor_tensor(
                ctx16[qc][pr:pr + 64, bc0:bc0 + S],
                cp[:DH, :], bc, op=OP.mult,
            )

        if "a" in phases:
            prev = None
            for i in range(len(pairs)):
                et = emit_scores_exp(i)
                if prev is not None:
                    emit_ctx(i - 1, prev)
                prev = et
            emit_ctx(len(pairs) - 1, prev)

    # ---------------- phase PROJ ----------------
    with tc.tile_pool(name="wp16p", bufs=1) as wp16p:
        wp16 = [wp16p.tile([128, E], F16, tag=f"wp{i}", name=f"wp{i}") for i in range(6)]
        for ec in range(6):
            nc.sync.dma_start(wp16[ec], wproj_d[ec * 128:(ec + 1) * 128, :])

        with tc.tile_pool(name="ppsum", bufs=4, space="PSUM") as ppsum, \
             tc.tile_pool(name="ostage", bufs=3) as ostage:
            for (off, sz) in (S_TILES if "p" in phases else []):
                pps = []
                for half in range(2):
                    pp = ppsum.tile([128, 384], F32, tag="pp")
                    nc.tensor.matmul(
                        pp[:sz],
                        ones16[:1, :sz],
                        bp16[:1, half * 384:(half + 1) * 384],
                        start=True, stop=False,
                    )
                    pps.append(pp)
                for ec in range(6):
                    for half in range(2):
                        nc.tensor.matmul(
                            pps[half][:sz],
                            ctx16[ec][:, off:off + sz],
                            wp16[ec][:, half * 384:(half + 1) * 384],
                            start=False, stop=(ec == 5),
                        )
                ot = ostage.tile([128, E], F32, tag="ot")
                for half in range(2):
                    nc.vector.tensor_copy(
                        ot[:sz, half * 384:(half + 1) * 384], pps[half][:sz]
                    )
                nc.sync.dma_start(out_d[off:off + sz, :], ot[:sz])


class ExitStackCompat:
    def __init__(self):
        from contextlib import ExitStack
        self._s = ExitStack()

    def __enter__(self):
        self._s.__enter__()
        return self._s

    def __exit__(self, *a):
        return self._s.__exit__(*a)


_NC_CACHE = {}


def get_nc(n_iters: int = 1):
    if n_iters not in _NC_CACHE:
        _NC_CACHE[n_iters] = build_bass(n_iters)
    return _NC_CACHE[n_iters]


def make_in_maps(inputs):
    x = np.asarray(inputs["x"], dtype=np.float32).astype(np.float16)
    wqkv = np.ascontiguousarray(
        np.asarray(inputs["W_qkv"], dtype=np.float32).astype(np.float16))
    bqkv = np.ascontiguousarray(np.asarray(inputs["b_qkv"], dtype=np.float32))
    bv16 = bqkv[2 * E:].astype(np.float16)[None, :]
    wproj = np.ascontiguousarray(
        np.asarray(inputs["W_proj"], dtype=np.float32).astype(np.float16))
    bp16 = np.asarray(inputs["b_proj"], dtype=np.float32).astype(
        np.float16)[None, :]
    cos_full, sin_full, pmat = _host_tables()
    in_maps = []
    for c in range(N_CORES):
        xs = np.ascontiguousarray(
            x[c * BPC:(c + 1) * BPC].reshape(T, E)
        )
        in_maps.append({
            "x": xs, "wqkv": wqkv, "bqkv": bqkv, "bv16": bv16,
            "wproj": wproj, "bp16": bp16,
            "cosT": cos_full, "sinT": sin_full, "pmat": pmat,
        })
    return in_maps


def kernel(**inputs) -> np.ndarray:
    nc = get_nc(1)
    in_maps = make_in_maps(inputs)
    res = run_bass_kernel_spmd(nc, in_maps, list(range(N_CORES)))
    outs = [res.results[c]["out"].reshape(BPC, S, E) for c in range(N_CORES)]
    return np.concatenate(outs, axis=0)

